# revision 6
# baseline (speedup 1.0000x reference)
"""Trainium2 Bass kernel for nn_AttentionHead (B=8, S=2048, H=1024, D=64).

Strategy: data-parallel over batch -- one batch element per NeuronCore,
8 cores, no collectives.  Per core, attention is computed in "transposed
space" so that no large on-device transposes of activations are needed:

  - host passes query/key/value pre-transposed as [H, S] and the relative
    bias pre-transposed as [Sk, Sq] (cheap strided numpy copies);
  - projections on PE produce qT/kT/vT [64, S] directly;
  - scoresT[sk, sq] = kT.T-slice @ qT (contraction over d=64 on partitions);
    q/k are duplicated into partitions 64-127 so two sk-tiles run
    concurrently in the two PE row-group halves (K=64 packing);
  - the relative bias is accumulated into the scores PSUM with an
    identity-weight matmul (no DVE pass);
  - exp on the scalar engine, no max-subtraction (logits are ~N(0,1),
    mathematically equivalent, overflow-impossible);
  - softmax denominator comes for free from a ones-column appended to V
    (AV matmul also contracts the ones row -> row sums);
  - the {0,1} key mask is folded multiplicatively into V rows (+ ones col),
    exactly reproducing masked_fill(-inf) semantics;
  - out.T [65, S] accumulates in PSUM over sk; final PE transpose back to
    [S, 65], divide by the denominator column, DMA out.

Compute dtype is fp16: all values here are O(10), so fp16's 10-bit
mantissa beats bf16 by ~8x in accuracy at identical PE/DMA cost.
"""

import os
from contextlib import ExitStack

import numpy as np

import concourse.bass as bass
import concourse.tile as tile
from concourse import bacc, mybir
from concourse.bass_utils import run_bass_kernel_spmd
from concourse.masks import make_identity

B, S, H, D = 8, 2048, 1024, 64
N_CORES = 8
FP = mybir.dt.float32

DTYPE_MODE = os.environ.get("KERNEL_DTYPE", "f16")
CD = {"f32": mybir.dt.float32, "bf16": mybir.dt.bfloat16,
      "f16": mybir.dt.float16}[DTYPE_MODE]

SQ_BLK = 1024  # sq columns per outer block
NT = S // SQ_BLK
NK = S // 128  # sk tiles
NH = H // 128  # hidden chunks


def _np_cd():
    if DTYPE_MODE == "bf16":
        import ml_dtypes

        return ml_dtypes.bfloat16
    if DTYPE_MODE == "f16":
        return np.float16
    return np.float32


def build_bass():
    nc = bacc.Bacc("TRN2", target_bir_lowering=False, debug=False,
                   num_devices=N_CORES)

    xqT = nc.dram_tensor("xqT", [H, S], CD, kind="ExternalInput").ap()
    xkT = nc.dram_tensor("xkT", [H, S], CD, kind="ExternalInput").ap()
    xvT = nc.dram_tensor("xvT", [H, S], CD, kind="ExternalInput").ap()
    biasT = nc.dram_tensor("biasT", [S, S], CD, kind="ExternalInput").ap()
    maskT = nc.dram_tensor("maskT", [128, NK], FP, kind="ExternalInput").ap()
    # weights pre-laid out as the SBUF image: [128, NH*D]
    wqT = nc.dram_tensor("wqT", [128, NH * D], CD, kind="ExternalInput").ap()
    wkT = nc.dram_tensor("wkT", [128, NH * D], CD, kind="ExternalInput").ap()
    wvT = nc.dram_tensor("wvT", [128, NH * D], CD, kind="ExternalInput").ap()
    bq = nc.dram_tensor("bq", [D, 1], FP, kind="ExternalInput").ap()
    bk = nc.dram_tensor("bk", [D, 1], FP, kind="ExternalInput").ap()
    bv = nc.dram_tensor("bv", [D, 1], FP, kind="ExternalInput").ap()
    out_d = nc.dram_tensor("out", [S, D], FP, kind="ExternalOutput").ap()

    with tile.TileContext(nc) as tc, ExitStack() as ctx:
        const = ctx.enter_context(tc.tile_pool(name="const", bufs=1))
        xin = ctx.enter_context(tc.tile_pool(name="xin", bufs=3))
        bias_in = ctx.enter_context(tc.tile_pool(name="bias_in", bufs=6))
        att_pool = ctx.enter_context(tc.tile_pool(name="att", bufs=4))
        avsb_pool = ctx.enter_context(tc.tile_pool(name="avsb", bufs=2))
        fin_pool = ctx.enter_context(tc.tile_pool(name="fin", bufs=4))
        # PSUM budget: psA slots [128,1024] f32 = 2 banks x3 = 6 banks
        # (projection accum, scores, small transposes); psB [65,512] = 1 bank
        # x2 = 2 banks (AV accumulators).  Total exactly 8 banks.
        psA = ctx.enter_context(tc.tile_pool(name="psA", bufs=3, space="PSUM"))
        psB = ctx.enter_context(tc.tile_pool(name="psB", bufs=2, space="PSUM"))

        ident = const.tile([128, 128], FP, tag="ident")
        make_identity(nc, ident)
        if CD != FP:
            ident_c = const.tile([128, 128], CD, tag="ident_c")
            nc.vector.tensor_copy(ident_c, ident)
        else:
            ident_c = ident

        w_sb = {}
        for name, wT in (("k", wkT), ("q", wqT), ("v", wvT)):
            w = const.tile([128, NH, D], CD, tag=f"w{name}")
            nc.sync.dma_start(out=w.rearrange("p t d -> p (t d)"), in_=wT)
            w_sb[name] = w
        b_sb = {}
        for name, bT in (("k", bk), ("q", bq), ("v", bv)):
            b = const.tile([D, 1], FP, tag=f"b{name}")
            nc.sync.dma_start(out=b, in_=bT)
            b_sb[name] = b
        mask_sb = const.tile([128, NK], FP, tag="mask")
        nc.sync.dma_start(out=mask_sb, in_=maskT)

        # q/k duplicated into both partition halves for K=64 row-packing
        KT2 = const.tile([128, S], CD, tag="KT2")
        QT2 = const.tile([128, S], CD, tag="QT2")
        vT_sb = const.tile([D, S], FP, tag="vT")
        v_aug = const.tile([128, NK, D + 1], CD, tag="v_aug")

        # ---- projections (k, then q, then v) ----
        for name, xT in (("k", xkT), ("q", xqT), ("v", xvT)):
            ps = [psA.tile([D, 1024], FP, tag="psA", name=f"proj_{name}_{i}")
                  for i in range(2)]
            for h in range(NH):
                x_sb = xin.tile([128, S], CD, tag="x", name=f"x_{name}_{h}")
                nc.sync.dma_start(out=x_sb, in_=xT[h * 128:(h + 1) * 128, :])
                for n in range(4):
                    nc.tensor.matmul(
                        ps[n // 2][:, (n % 2) * 512:(n % 2) * 512 + 512],
                        lhsT=w_sb[name][:, h, :],
                        rhs=x_sb[:, n * 512:(n + 1) * 512],
                        start=(h == 0), stop=(h == NH - 1))
            for i in range(2):
                cols = slice(i * 1024, (i + 1) * 1024)
                if name == "v":
                    nc.vector.tensor_scalar_add(out=vT_sb[:, cols], in0=ps[i],
                                                scalar1=b_sb[name])
                else:
                    dst = KT2 if name == "k" else QT2
                    nc.vector.tensor_scalar_add(out=dst[0:D, cols], in0=ps[i],
                                                scalar1=b_sb[name])
                    nc.vector.tensor_scalar_add(out=dst[D:2 * D, cols],
                                                in0=ps[i], scalar1=b_sb[name])

        # v_aug[p, sk, :D] = vT.T rows scaled by mask; col D = mask
        for sk in range(NK):
            vt = psA.tile([128, D], FP, tag="psA", name=f"vt_{sk}")
            nc.tensor.matmul(vt, lhsT=vT_sb[:, sk * 128:(sk + 1) * 128],
                             rhs=ident[:D, :D], is_transpose=True)
            nc.vector.tensor_scalar_mul(out=v_aug[:, sk, 0:D], in0=vt,
                                        scalar1=mask_sb[:, sk:sk + 1])
            nc.vector.tensor_copy(out=v_aug[:, sk, D:D + 1],
                                  in_=mask_sb[:, sk:sk + 1])

        # ---- attention ----
        inv_sqrt_d = 1.0 / np.sqrt(float(D))
        for nt in range(NT):
            sq0 = nt * SQ_BLK
            av = [psB.tile([D + 1, 512], FP, tag="psB", name=f"av_{nt}_{i}")
                  for i in range(2)]
            for skp in range(NK // 2):
                sks = (2 * skp, 2 * skp + 1)
                bias_t, sc = [], []
                for j, sk in enumerate(sks):
                    bt = bias_in.tile([128, SQ_BLK], CD, tag="bias",
                                      name=f"bias_{nt}_{sk}")
                    nc.sync.dma_start(
                        out=bt,
                        in_=biasT[sk * 128:(sk + 1) * 128, sq0:sq0 + SQ_BLK])
                    bias_t.append(bt)
                    sc.append(psA.tile([128, SQ_BLK], FP, tag="psA",
                                       name=f"sc_{nt}_{sk}"))
                # packed K=64 score matmuls: row groups 0-63 / 64-127 run
                # concurrently in the PE array halves
                for i in range(SQ_BLK // 512):
                    cols = slice(i * 512, (i + 1) * 512)
                    for j, sk in enumerate(sks):
                        rows = slice(j * D, (j + 1) * D)
                        nc.tensor.matmul(
                            sc[j][:, cols],
                            lhsT=KT2[rows, sk * 128:(sk + 1) * 128],
                            rhs=QT2[rows, sq0 + i * 512:sq0 + (i + 1) * 512],
                            start=True, stop=False)
                    for j in range(2):
                        nc.tensor.matmul(sc[j][:, cols], lhsT=ident_c,
                                         rhs=bias_t[j][:, cols],
                                         start=False, stop=True)
                for j, sk in enumerate(sks):
                    att = att_pool.tile([128, SQ_BLK], CD, tag="att",
                                        name=f"att_{nt}_{sk}")
                    nc.scalar.activation(out=att, in_=sc[j],
                                         func=mybir.ActivationFunctionType.Exp,
                                         scale=inv_sqrt_d)
                    for i in range(SQ_BLK // 512):
                        cols = slice(i * 512, (i + 1) * 512)
                        nc.tensor.matmul(av[i], lhsT=v_aug[:, sk, :],
                                         rhs=att[:, cols],
                                         start=(sk == 0), stop=(sk == NK - 1))
            avs = avsb_pool.tile([D + 1, SQ_BLK], FP, tag="avs",
                                 name=f"avs_{nt}")
            for i in range(2):
                nc.vector.tensor_copy(out=avs[:, i * 512:(i + 1) * 512],
                                      in_=av[i])
            for j in range(SQ_BLK // 128):
                ot = psA.tile([128, D + 1], FP, tag="psA", name=f"ot_{nt}_{j}")
                nc.tensor.matmul(ot, lhsT=avs[:, j * 128:(j + 1) * 128],
                                 rhs=ident[:D + 1, :D + 1], is_transpose=True)
                rec = fin_pool.tile([128, 1], FP, tag="rec",
                                    name=f"rec_{nt}_{j}")
                nc.vector.reciprocal(out=rec, in_=ot[:, D:D + 1])
                fin = fin_pool.tile([128, D], FP, tag="fin",
                                    name=f"fin_{nt}_{j}")
                nc.vector.tensor_scalar_mul(out=fin, in0=ot[:, 0:D],
                                            scalar1=rec)
                r0 = sq0 + j * 128
                nc.sync.dma_start(out=out_d[r0:r0 + 128, :], in_=fin)

    nc.compile()
    return nc


_NC = None


def _get_nc():
    global _NC
    if _NC is None:
        _NC = build_bass()
    return _NC


def _prep_core_inputs(b, query, key, value, relative_biases, mask,
                      Wq, bq, Wk, bk, Wv, bv):
    cd = _np_cd()

    def wprep(W):
        # SBUF image [128, NH*D]: (p, t*D+d) = W.T[t*128+p, d]
        return np.ascontiguousarray(
            W.T.astype(cd).reshape(NH, 128, D).transpose(1, 0, 2).reshape(
                128, NH * D))

    return {
        "xqT": np.ascontiguousarray(query[b].T.astype(cd, copy=False)),
        "xkT": np.ascontiguousarray(key[b].T.astype(cd, copy=False)),
        "xvT": np.ascontiguousarray(value[b].T.astype(cd, copy=False)),
        "biasT": np.ascontiguousarray(
            relative_biases[b].T.astype(cd, copy=False)),
        "maskT": np.ascontiguousarray(
            mask[b].astype(np.float32).reshape(NK, 128).T),
        "wqT": wprep(Wq),
        "wkT": wprep(Wk),
        "wvT": wprep(Wv),
        "bq": np.asarray(bq, np.float32).reshape(D, 1),
        "bk": np.asarray(bk, np.float32).reshape(D, 1),
        "bv": np.asarray(bv, np.float32).reshape(D, 1),
    }


def kernel(query, key, value, relative_biases, mask, Wq, bq, Wk, bk, Wv, bv):
    query = np.asarray(query, np.float32)
    key = np.asarray(key, np.float32)
    value = np.asarray(value, np.float32)
    relative_biases = np.asarray(relative_biases, np.float32)
    mask = np.asarray(mask)
    Wq, Wk, Wv = (np.asarray(w, np.float32) for w in (Wq, Wk, Wv))

    nc = _get_nc()
    in_maps = [
        _prep_core_inputs(b, query, key, value, relative_biases, mask,
                          Wq, bq, Wk, bk, Wv, bv)
        for b in range(B)
    ]
    res = run_bass_kernel_spmd(nc, in_maps, core_ids=list(range(N_CORES)))
    out = np.stack([res.results[i]["out"] for i in range(N_CORES)], axis=0)
    return out.astype(np.float32)


# revision 7
# speedup vs baseline: 1.0372x; 1.0372x over previous
"""Trainium2 Bass kernel for nn_AttentionHead (B=8, S=2048, H=1024, D=64).

Strategy: data-parallel over batch -- one batch element per NeuronCore,
8 cores, no collectives.  Per core, attention is computed in "transposed
space" so that no large on-device transposes of activations are needed:

  - host passes query/key/value pre-transposed as [H, S] and the relative
    bias pre-transposed as [Sk, Sq] (cheap strided numpy copies);
  - projections on PE produce qT/kT/vT [64, S] directly;
  - scoresT[sk, sq] = kT.T-slice @ qT (contraction over d=64 on partitions);
    q/k are duplicated into partitions 64-127 so two sk-tiles run
    concurrently in the two PE row-group halves (K=64 packing);
  - the relative bias is accumulated into the scores PSUM with an
    identity-weight matmul (no DVE pass);
  - exp on the scalar engine, no max-subtraction (logits are ~N(0,1),
    mathematically equivalent, overflow-impossible);
  - softmax denominator comes for free from a ones-column appended to V
    (AV matmul also contracts the ones row -> row sums);
  - the {0,1} key mask is folded multiplicatively into V rows (+ ones col),
    exactly reproducing masked_fill(-inf) semantics;
  - out.T [65, S] accumulates in PSUM over sk; final PE transpose back to
    [S, 65], divide by the denominator column, DMA out.

Compute dtype is fp16: all values here are O(10), so fp16's 10-bit
mantissa beats bf16 by ~8x in accuracy at identical PE/DMA cost.
"""

import os
from contextlib import ExitStack

import numpy as np

import concourse.bass as bass
import concourse.tile as tile
from concourse import bacc, mybir
from concourse.bass_utils import run_bass_kernel_spmd
from concourse.masks import make_identity

B, S, H, D = 8, 2048, 1024, 64
N_CORES = 8
FP = mybir.dt.float32

DTYPE_MODE = os.environ.get("KERNEL_DTYPE", "f16")
CD = {"f32": mybir.dt.float32, "bf16": mybir.dt.bfloat16,
      "f16": mybir.dt.float16}[DTYPE_MODE]

SQ_BLK = 1024  # sq columns per outer block
NT = S // SQ_BLK
NK = S // 128  # sk tiles
NH = H // 128  # hidden chunks


def _np_cd():
    if DTYPE_MODE == "bf16":
        import ml_dtypes

        return ml_dtypes.bfloat16
    if DTYPE_MODE == "f16":
        return np.float16
    return np.float32


def build_bass():
    nc = bacc.Bacc("TRN2", target_bir_lowering=False, debug=False,
                   num_devices=N_CORES)

    xqT = nc.dram_tensor("xqT", [H, S], CD, kind="ExternalInput").ap()
    xkT = nc.dram_tensor("xkT", [H, S], CD, kind="ExternalInput").ap()
    xvT = nc.dram_tensor("xvT", [H, S], CD, kind="ExternalInput").ap()
    biasT = nc.dram_tensor("biasT", [S, S], CD, kind="ExternalInput").ap()
    maskT = nc.dram_tensor("maskT", [128, NK], FP, kind="ExternalInput").ap()
    # weights pre-laid out as the SBUF image: [128, NH*D]
    wqT = nc.dram_tensor("wqT", [128, NH * D], CD, kind="ExternalInput").ap()
    wkT = nc.dram_tensor("wkT", [128, NH * D], CD, kind="ExternalInput").ap()
    wvT = nc.dram_tensor("wvT", [128, NH * D], CD, kind="ExternalInput").ap()
    bq = nc.dram_tensor("bq", [D, 1], FP, kind="ExternalInput").ap()
    bk = nc.dram_tensor("bk", [D, 1], FP, kind="ExternalInput").ap()
    bv = nc.dram_tensor("bv", [D, 1], FP, kind="ExternalInput").ap()
    out_d = nc.dram_tensor("out", [S, D], FP, kind="ExternalOutput").ap()

    with tile.TileContext(nc) as tc, ExitStack() as ctx:
        const = ctx.enter_context(tc.tile_pool(name="const", bufs=1))
        xin = ctx.enter_context(tc.tile_pool(name="xin", bufs=3))
        bias_in = ctx.enter_context(tc.tile_pool(name="bias_in", bufs=5))
        att_pool = ctx.enter_context(tc.tile_pool(name="att", bufs=4))
        avsb_pool = ctx.enter_context(tc.tile_pool(name="avsb", bufs=2))
        fin_pool = ctx.enter_context(tc.tile_pool(name="fin", bufs=4))
        # PSUM budget: psA slots [128,1024] f32 = 2 banks x3 = 6 banks
        # (projection accum, scores, small transposes); psB [65,512] = 1 bank
        # x2 = 2 banks (AV accumulators).  Total exactly 8 banks.
        psA = ctx.enter_context(tc.tile_pool(name="psA", bufs=3, space="PSUM"))
        psB = ctx.enter_context(tc.tile_pool(name="psB", bufs=2, space="PSUM"))

        ident = const.tile([128, 128], FP, tag="ident")
        make_identity(nc, ident)
        if CD != FP:
            ident_c = const.tile([128, 128], CD, tag="ident_c")
            nc.vector.tensor_copy(ident_c, ident)
        else:
            ident_c = ident

        w_sb = {}
        for name, wT in (("k", wkT), ("q", wqT), ("v", wvT)):
            w = const.tile([128, NH, D], CD, tag=f"w{name}")
            nc.sync.dma_start(out=w.rearrange("p t d -> p (t d)"), in_=wT)
            w_sb[name] = w
        b_sb = {}
        for name, bT in (("k", bk), ("q", bq), ("v", bv)):
            b = const.tile([D, 1], FP, tag=f"b{name}")
            nc.sync.dma_start(out=b, in_=bT)
            b_sb[name] = b
        mask_sb = const.tile([128, NK], FP, tag="mask")
        nc.sync.dma_start(out=mask_sb, in_=maskT)

        # q/k duplicated into both partition halves for K=64 row-packing
        KT2 = const.tile([128, S], CD, tag="KT2")
        QT2 = const.tile([128, S], CD, tag="QT2")
        vT_sb = const.tile([D, S], FP, tag="vT")
        v_aug = const.tile([128, NK, D + 1], CD, tag="v_aug")

        # ---- projections (k, then q, then v) ----
        for name, xT in (("k", xkT), ("q", xqT), ("v", xvT)):
            ps = [psA.tile([D, 1024], FP, tag="psA", name=f"proj_{name}_{i}")
                  for i in range(2)]
            for hp in range(NH // 2):
                x_sb = xin.tile([128, 2, S], CD, tag="x", name=f"x_{name}_{hp}")
                nc.sync.dma_start(
                    out=x_sb,
                    in_=xT[hp * 256:(hp + 1) * 256, :].rearrange(
                        "(j p) s -> p j s", p=128))
                for jj in range(2):
                    h = 2 * hp + jj
                    for n in range(4):
                        nc.tensor.matmul(
                            ps[n // 2][:, (n % 2) * 512:(n % 2) * 512 + 512],
                            lhsT=w_sb[name][:, h, :],
                            rhs=x_sb[:, jj, n * 512:(n + 1) * 512],
                            start=(h == 0), stop=(h == NH - 1))
            for i in range(2):
                cols = slice(i * 1024, (i + 1) * 1024)
                if name == "v":
                    nc.vector.tensor_scalar_add(out=vT_sb[:, cols], in0=ps[i],
                                                scalar1=b_sb[name])
                else:
                    dst = KT2 if name == "k" else QT2
                    nc.vector.tensor_scalar_add(out=dst[0:D, cols], in0=ps[i],
                                                scalar1=b_sb[name])
                    nc.vector.tensor_scalar_add(out=dst[D:2 * D, cols],
                                                in0=ps[i], scalar1=b_sb[name])

        # v_aug[p, sk, :D] = vT.T rows scaled by mask; col D = mask
        for sk in range(NK):
            vt = psA.tile([128, D], FP, tag="psA", name=f"vt_{sk}")
            nc.tensor.matmul(vt, lhsT=vT_sb[:, sk * 128:(sk + 1) * 128],
                             rhs=ident[:D, :D], is_transpose=True)
            nc.vector.tensor_scalar_mul(out=v_aug[:, sk, 0:D], in0=vt,
                                        scalar1=mask_sb[:, sk:sk + 1])
            nc.vector.tensor_copy(out=v_aug[:, sk, D:D + 1],
                                  in_=mask_sb[:, sk:sk + 1])

        # ---- attention ----
        inv_sqrt_d = 1.0 / np.sqrt(float(D))
        for nt in range(NT):
            sq0 = nt * SQ_BLK
            av = [psB.tile([D + 1, 512], FP, tag="psB", name=f"av_{nt}_{i}")
                  for i in range(2)]
            for skp in range(NK // 2):
                sks = (2 * skp, 2 * skp + 1)
                if skp % 2 == 0:
                    bias_g = bias_in.tile([128, 4, SQ_BLK], CD, tag="bias",
                                          name=f"bias_{nt}_{skp}")
                    sk0 = sks[0]
                    nc.sync.dma_start(
                        out=bias_g,
                        in_=biasT[sk0 * 128:(sk0 + 4) * 128,
                                  sq0:sq0 + SQ_BLK].rearrange(
                            "(j p) c -> p j c", p=128))
                bias_t, sc = [], []
                for j, sk in enumerate(sks):
                    bias_t.append(bias_g[:, 2 * (skp % 2) + j, :])
                    sc.append(psA.tile([128, SQ_BLK], FP, tag="psA",
                                       name=f"sc_{nt}_{sk}"))
                # packed K=64 score matmuls: row groups 0-63 / 64-127 run
                # concurrently in the PE array halves
                for i in range(SQ_BLK // 512):
                    cols = slice(i * 512, (i + 1) * 512)
                    for j, sk in enumerate(sks):
                        rows = slice(j * D, (j + 1) * D)
                        nc.tensor.matmul(
                            sc[j][:, cols],
                            lhsT=KT2[rows, sk * 128:(sk + 1) * 128],
                            rhs=QT2[rows, sq0 + i * 512:sq0 + (i + 1) * 512],
                            start=True, stop=False)
                    for j in range(2):
                        nc.tensor.matmul(sc[j][:, cols], lhsT=ident_c,
                                         rhs=bias_t[j][:, cols],
                                         start=False, stop=True)
                for j, sk in enumerate(sks):
                    att = att_pool.tile([128, SQ_BLK], CD, tag="att",
                                        name=f"att_{nt}_{sk}")
                    nc.scalar.activation(out=att, in_=sc[j],
                                         func=mybir.ActivationFunctionType.Exp,
                                         scale=inv_sqrt_d)
                    for i in range(SQ_BLK // 512):
                        cols = slice(i * 512, (i + 1) * 512)
                        nc.tensor.matmul(av[i], lhsT=v_aug[:, sk, :],
                                         rhs=att[:, cols],
                                         start=(sk == 0), stop=(sk == NK - 1))
            avs = avsb_pool.tile([D + 1, SQ_BLK], FP, tag="avs",
                                 name=f"avs_{nt}")
            for i in range(2):
                nc.vector.tensor_copy(out=avs[:, i * 512:(i + 1) * 512],
                                      in_=av[i])
            for j in range(SQ_BLK // 128):
                ot = psA.tile([128, D + 1], FP, tag="psA", name=f"ot_{nt}_{j}")
                nc.tensor.matmul(ot, lhsT=avs[:, j * 128:(j + 1) * 128],
                                 rhs=ident[:D + 1, :D + 1], is_transpose=True)
                rec = fin_pool.tile([128, 1], FP, tag="rec",
                                    name=f"rec_{nt}_{j}")
                nc.vector.reciprocal(out=rec, in_=ot[:, D:D + 1])
                fin = fin_pool.tile([128, D], FP, tag="fin",
                                    name=f"fin_{nt}_{j}")
                nc.vector.tensor_scalar_mul(out=fin, in0=ot[:, 0:D],
                                            scalar1=rec)
                r0 = sq0 + j * 128
                nc.sync.dma_start(out=out_d[r0:r0 + 128, :], in_=fin)

    nc.compile()
    return nc


_NC = None


def _get_nc():
    global _NC
    if _NC is None:
        _NC = build_bass()
    return _NC


def _prep_core_inputs(b, query, key, value, relative_biases, mask,
                      Wq, bq, Wk, bk, Wv, bv):
    cd = _np_cd()

    def wprep(W):
        # SBUF image [128, NH*D]: (p, t*D+d) = W.T[t*128+p, d]
        return np.ascontiguousarray(
            W.T.astype(cd).reshape(NH, 128, D).transpose(1, 0, 2).reshape(
                128, NH * D))

    return {
        "xqT": np.ascontiguousarray(query[b].T.astype(cd, copy=False)),
        "xkT": np.ascontiguousarray(key[b].T.astype(cd, copy=False)),
        "xvT": np.ascontiguousarray(value[b].T.astype(cd, copy=False)),
        "biasT": np.ascontiguousarray(
            relative_biases[b].T.astype(cd, copy=False)),
        "maskT": np.ascontiguousarray(
            mask[b].astype(np.float32).reshape(NK, 128).T),
        "wqT": wprep(Wq),
        "wkT": wprep(Wk),
        "wvT": wprep(Wv),
        "bq": np.asarray(bq, np.float32).reshape(D, 1),
        "bk": np.asarray(bk, np.float32).reshape(D, 1),
        "bv": np.asarray(bv, np.float32).reshape(D, 1),
    }


def kernel(query, key, value, relative_biases, mask, Wq, bq, Wk, bk, Wv, bv):
    query = np.asarray(query, np.float32)
    key = np.asarray(key, np.float32)
    value = np.asarray(value, np.float32)
    relative_biases = np.asarray(relative_biases, np.float32)
    mask = np.asarray(mask)
    Wq, Wk, Wv = (np.asarray(w, np.float32) for w in (Wq, Wk, Wv))

    nc = _get_nc()
    in_maps = [
        _prep_core_inputs(b, query, key, value, relative_biases, mask,
                          Wq, bq, Wk, bk, Wv, bv)
        for b in range(B)
    ]
    res = run_bass_kernel_spmd(nc, in_maps, core_ids=list(range(N_CORES)))
    out = np.stack([res.results[i]["out"] for i in range(N_CORES)], axis=0)
    return out.astype(np.float32)
